# revision 1
# baseline (speedup 1.0000x reference)
"""ARMSNorm (int8 fake-quant RMS norm) Trainium2 kernel, 8-way data parallel.

Layout: x (4,4096,2048) f32 -> rows 16384 x 2048; core c owns rows
[c*2048, (c+1)*2048). Per core, the 16 MiB shard stays resident in SBUF:

  phase A: DMA in (1 MiB chunks) + per-row absmax (DVE reduce) -> local max
           -> AllGather(8) -> global max -> scale_in = max(gmax/127, 1e-8)
  phase B: x_int = round(x*inv_s) as int8 (DVE f32->int conversion is
           round-to-nearest-even, matching jnp.round incl. ties);
           ACT Square with accum_out gives exact integer row sums of x_int^2;
           var = clip(round(sum*scale_in^2/2048), 1, 65535);
           std = round(sqrt(var)) exactly as 1 + #[var > q^2+q] via
           tensor_tensor is_gt + reduce, interleaved per 4-column group in
           the DVE slack under the ACT-bound squares, along with the whole
           inv_std/row-ymax chain;
           row ymax = round(rowmax|x|*inv_s)*scale_in*inv_std*|w|
           -> AllGather(8) -> scale_out = max(ymax/127, 1e-8)
  phase C: q = round(x_int * k_row) as int16 (k_row = scale_in*inv_std*w
           /scale_out); y = q*scale_out on ACT; 2 MiB DMA-out chunks.

AllGather results are DMA'd back with a partition-replicating access
pattern so the scalar chains run on all 128 partitions directly.

HBM traffic per core: 16 MiB in + 16 MiB out (minimal: every element is
read once and written once).
"""

from contextlib import ExitStack

import numpy as np

import concourse.bacc as bacc
import concourse.bass as bass
import concourse.bass_isa as bass_isa
import concourse.mybir as mybir
import concourse.tile as tile
from concourse import bass_utils

N_CORES = 8
P = 128
Q = 255  # sqrt boundary table size (covers var up to 65535)

_cache: dict = {}


def _emit(nc, tc, x_dram, btab_dram, y_dram, w0: float, rows_per_core: int, d: int,
          wt_dram=None):
    f32, i32 = mybir.dt.float32, mybir.dt.int32
    i8, i16, bf16 = mybir.dt.int8, mybir.dt.int16, mybir.dt.bfloat16
    OP = mybir.AluOpType
    AX = mybir.AxisListType.X
    AF = mybir.ActivationFunctionType
    T = rows_per_core // P          # 128-row (1 MiB) blocks
    T2 = T // 2                     # 256-row (2 MiB) output chunks
    RG = [list(range(N_CORES))]
    x_ap = x_dram.ap()
    y_ap = y_dram.ap()

    def allgather(dr, pool, name, src_pP1):
        """[P,1] per-core scalar -> [P,1] tile with the global max on every
        partition. partition_all_reduce + AllGather + replicating DMA-back."""
        pr = pool.tile([P, 1], f32, name=f"{name}_pr")
        nc.gpsimd.partition_all_reduce(pr[:], src_pP1, channels=P,
                                       reduce_op=bass_isa.ReduceOp.max)
        ag_in = dr.tile([1, 1], f32, name=f"{name}_in")
        ag_out = dr.tile([N_CORES, 1], f32, name=f"{name}_out",
                         addr_space="Shared")
        nc.sync.dma_start(ag_in[:], pr[:1, :])
        nc.gpsimd.collective_compute("AllGather", OP.bypass, replica_groups=RG,
                                     ins=[ag_in[:]], outs=[ag_out[:]])
        rep = pool.tile([P, N_CORES], f32, name=f"{name}_rep")
        nc.sync.dma_start(
            rep[:], ag_out[:].rearrange("e one -> one e").broadcast_to([P, N_CORES]))
        gmx = pool.tile([P, 1], f32, name=f"{name}_gmx")
        nc.vector.tensor_reduce(out=gmx[:], in_=rep[:], axis=AX, op=OP.max)
        return gmx

    with (
        tc.tile_pool(name="st", bufs=1) as st,
        tc.tile_pool(name="m8p", bufs=T) as m8p,
        tc.tile_pool(name="pp", bufs=2, space="PSUM") as pp,
        tc.tile_pool(name="dram", bufs=1, space="DRAM") as dr,
    ):
        # ---- stats buffers
        rowmax = st.tile([P, T], f32, name="rowmax")
        sums = st.tile([P, T], f32, name="sums")
        btab = st.tile([P, Q], f32, name="btab")

        with ExitStack() as xstack:
            xp = xstack.enter_context(tc.tile_pool(name="xp", bufs=T))

            # ---- phase A: load (1 MiB chunks) + per-row absmax
            x_t = []
            for t in range(T):
                xt = xp.tile([P, d], f32, name=f"x{t}", tag="x")
                x_t.append(xt)
                nc.sync.dma_start(xt[:], x_ap[t * P:(t + 1) * P, :])
                nc.vector.tensor_reduce(out=rowmax[:, t:t + 1], in_=xt[:],
                                        axis=AX, op=OP.max,
                                        apply_absolute_value=True)

            lmax = st.tile([P, 1], f32, name="lmax")
            nc.vector.tensor_reduce(out=lmax[:], in_=rowmax[:], axis=AX,
                                    op=OP.max)
            gmax = allgather(dr, st, "ag1", lmax[:])

            nc.sync.dma_start(btab[:], btab_dram.ap())
            if wt_dram is not None:
                wb = st.tile([P, d], f32, name="wb")
                nc.sync.dma_start(
                    wb[:], wt_dram.ap().broadcast_to([P, d]))

            # ---- scalar chain 1 (computed on all partitions)
            scale_in = st.tile([P, 1], f32, name="scale_in")
            nc.vector.tensor_scalar(out=scale_in[:], in0=gmax[:],
                                    scalar1=1.0 / 127.0, scalar2=1e-8,
                                    op0=OP.mult, op1=OP.max)
            inv_s = st.tile([P, 1], f32, name="inv_s")
            nc.vector.reciprocal(inv_s[:], scale_in[:])
            sc2 = st.tile([P, 1], f32, name="sc2")
            nc.vector.tensor_scalar(out=sc2[:], in0=scale_in[:],
                                    scalar1=scale_in[:], scalar2=1.0 / d,
                                    op0=OP.mult, op1=OP.mult)
            siw_s = st.tile([P, 1], f32, name="siw_s")
            nc.vector.tensor_scalar(out=siw_s[:], in0=scale_in[:],
                                    scalar1=abs(w0), scalar2=None, op0=OP.mult)

            # ---- phase B: quantize (RNE) + integer square row sums; the
            # full per-row stats chain interleaved per 4-column group in
            # the DVE slack under the ACT-bound squares.
            var = st.tile([P, T], i32, name="var")
            varc = st.tile([P, T], i32, name="varc")
            gt = st.tile([P, T, Q], bf16, name="gt")
            stdm1 = st.tile([P, T], f32, name="stdm1")
            std = st.tile([P, T], f32, name="std")
            inv_std = st.tile([P, T], f32, name="inv_std")
            rmx_i = st.tile([P, T], i32, name="rmx_i")
            if wt_dram is not None:
                wmax = st.tile([P, T], f32, name="wmax")
            siw = st.tile([P, T], f32, name="siw")
            ymr = st.tile([P, T], f32, name="ymr")
            m8_t = []
            ym_parts = []
            for t in range(T):
                m8 = m8p.tile([P, d], i8, name=f"m8{t}", tag="m8")
                m8_t.append(m8)
                nc.vector.tensor_scalar(out=m8[:], in0=x_t[t][:],
                                        scalar1=inv_s[:], scalar2=None,
                                        op0=OP.mult)
                dump = pp.tile([P, d], f32, name=f"dump{t}", tag="dump")
                nc.scalar.activation(dump[:], m8[:], AF.Square, bias=0.0,
                                     scale=1.0, accum_out=sums[:, t:t + 1])
                if wt_dram is not None:
                    mw_f = st.tile([P, d], f32, name=f"mw{t}", tag="mwf", bufs=2)
                    nc.vector.tensor_tensor(out=mw_f[:], in0=m8[:], in1=wb[:],
                                            op=OP.mult)
                    nc.vector.tensor_reduce(out=wmax[:, t:t + 1], in_=mw_f[:],
                                            axis=AX, op=OP.max,
                                            apply_absolute_value=True)
            # stats groups emitted after all m8 conversions: the DVE slack at
            # the end of the ACT-bound square phase absorbs them without
            # starving the ACT queue mid-phase.
            ends = [t + 1 for t in range(T)
                    if (t % 4 == 3 and t != T - 1) or t == T - 2 or t == T - 1]
            prev = 0
            for e in ends:
                cs = slice(prev, e)
                w = e - prev
                prev = e
                nc.vector.tensor_scalar(out=var[:, cs], in0=sums[:, cs],
                                        scalar1=sc2[:], scalar2=None,
                                        op0=OP.mult)
                nc.vector.tensor_scalar(out=varc[:, cs], in0=var[:, cs],
                                        scalar1=1.0, scalar2=65535.0,
                                        op0=OP.max, op1=OP.min)
                nc.vector.tensor_tensor(
                    out=gt[:, cs, :],
                    in0=varc[:, cs].rearrange(
                        "p t -> p t ()").broadcast_to([P, w, Q]),
                    in1=btab[:].rearrange(
                        "p q -> p () q").broadcast_to([P, w, Q]),
                    op=OP.is_gt)
                nc.vector.tensor_reduce(out=stdm1[:, cs], in_=gt[:, cs, :],
                                        axis=AX, op=OP.add)
                nc.vector.tensor_scalar(out=std[:, cs], in0=stdm1[:, cs],
                                        scalar1=1.0, scalar2=None,
                                        op0=OP.add)
                nc.vector.reciprocal(inv_std[:, cs], std[:, cs])
                nc.vector.tensor_scalar(out=siw[:, cs], in0=inv_std[:, cs],
                                        scalar1=siw_s[:], scalar2=None,
                                        op0=OP.mult)
                if wt_dram is None:
                    nc.vector.tensor_scalar(out=rmx_i[:, cs],
                                            in0=rowmax[:, cs],
                                            scalar1=inv_s[:], scalar2=None,
                                            op0=OP.mult)
                    nc.vector.tensor_tensor(out=ymr[:, cs], in0=rmx_i[:, cs],
                                            in1=siw[:, cs], op=OP.mult)
                else:
                    nc.vector.tensor_tensor(out=ymr[:, cs], in0=wmax[:, cs],
                                            in1=siw[:, cs], op=OP.mult)
                if e == 12 and T == 16:
                    yl_a = st.tile([P, 1], f32, name="yl_a")
                    nc.vector.tensor_reduce(out=yl_a[:], in_=ymr[:, :12],
                                            axis=AX, op=OP.max)
                    ym_parts.append(allgather(dr, st, "ag2a", yl_a[:]))

        # x pool released here; phase-C pools reuse its SBUF space.
        with (
            tc.tile_pool(name="qp", bufs=4) as qp,
            tc.tile_pool(name="yp", bufs=4) as yp,
        ):
            ymax_l = st.tile([P, 1], f32, name="ymax_l")
            if ym_parts:
                nc.vector.tensor_reduce(out=ymax_l[:], in_=ymr[:, 12:], axis=AX,
                                        op=OP.max)
                ym_b = allgather(dr, st, "ag2b", ymax_l[:])
                so_raw = st.tile([P, 1], f32, name="so_raw")
                nc.vector.tensor_scalar(out=so_raw[:], in0=ym_b[:],
                                        scalar1=ym_parts[0][:],
                                        scalar2=1.0 / 127.0,
                                        op0=OP.max, op1=OP.mult)
            else:
                nc.vector.tensor_reduce(out=ymax_l[:], in_=ymr[:], axis=AX,
                                        op=OP.max)
                ymax = allgather(dr, st, "ag2", ymax_l[:])
                so_raw = st.tile([P, 1], f32, name="so_raw")
                nc.vector.tensor_scalar(out=so_raw[:], in0=ymax[:],
                                        scalar1=1.0 / 127.0, scalar2=None,
                                        op0=OP.mult)

            # ---- scalar chain 2
            so_b = st.tile([P, 1], f32, name="so_b")
            nc.vector.tensor_scalar(out=so_b[:], in0=so_raw[:], scalar1=1e-8,
                                    scalar2=None, op0=OP.max)
            inv_so = st.tile([P, 1], f32, name="inv_so")
            nc.vector.reciprocal(inv_so[:], so_b[:])
            k0 = st.tile([P, 1], f32, name="k0")
            nc.vector.tensor_scalar(out=k0[:], in0=inv_so[:], scalar1=scale_in[:],
                                    scalar2=float(w0), op0=OP.mult, op1=OP.mult)
            k_row = st.tile([P, T], f32, name="k_row")
            nc.vector.tensor_scalar(out=k_row[:], in0=inv_std[:], scalar1=k0[:],
                                    scalar2=None, op0=OP.mult)

            # ---- phase C: requantize (RNE) + scale + output (1 MiB chunks)
            for t in range(T):
                q_t = qp.tile([P, d], i16, name=f"q{t}", tag="q")
                if wt_dram is None:
                    nc.vector.tensor_scalar(
                        out=q_t[:], in0=m8_t[t][:],
                        scalar1=k_row[:, t:t + 1], scalar2=None, op0=OP.mult)
                else:
                    mw_c = st.tile([P, d], f32, name=f"mwc{t}", tag="mwc",
                                   bufs=2)
                    nc.vector.tensor_tensor(out=mw_c[:], in0=m8_t[t][:],
                                            in1=wb[:], op=OP.mult)
                    nc.vector.tensor_scalar(
                        out=q_t[:], in0=mw_c[:],
                        scalar1=k_row[:, t:t + 1], scalar2=None, op0=OP.mult)
                y_t = yp.tile([P, d], f32, name=f"y{t}", tag="y")
                nc.vector.tensor_scalar(out=y_t[:], in0=q_t[:],
                                        scalar1=so_b[:], scalar2=None,
                                        op0=OP.mult)
                nc.sync.dma_start(y_ap[t * P:(t + 1) * P, :], y_t[:])


def _build(w0, rows_per_core: int, d: int, uniform: bool = True):
    nc = bacc.Bacc("TRN2", target_bir_lowering=False, debug=False,
                   num_devices=N_CORES)
    x_dram = nc.dram_tensor("x", [rows_per_core, d], mybir.dt.float32,
                            kind="ExternalInput")
    btab_dram = nc.dram_tensor("btab", [P, Q], mybir.dt.float32,
                               kind="ExternalInput")
    wt_dram = None
    if not uniform:
        wt_dram = nc.dram_tensor("wt", [1, d], mybir.dt.float32,
                                 kind="ExternalInput")
    y_dram = nc.dram_tensor("y", [rows_per_core, d], mybir.dt.float32,
                            kind="ExternalOutput")
    with tile.TileContext(nc) as tc:
        _emit(nc, tc, x_dram, btab_dram, y_dram,
              w0 if uniform else 1.0, rows_per_core, d, wt_dram=wt_dram)
    nc.compile()
    return nc


def _btab() -> np.ndarray:
    q = np.arange(1, Q + 1, dtype=np.int64)
    return np.tile((q * q + q).astype(np.float32), (P, 1))


def kernel(x: np.ndarray, weight: np.ndarray, _trace: bool = False):
    x = np.asarray(x, dtype=np.float32)
    weight = np.asarray(weight, dtype=np.float32)
    rows = int(np.prod(x.shape[:-1]))
    d = x.shape[-1]
    rows_per_core = rows // N_CORES
    uniform = bool(np.all(weight == weight[0]))
    w0 = float(weight[0])

    key = (w0 if uniform else None, rows_per_core, d)
    if key not in _cache:
        _cache[key] = _build(w0, rows_per_core, d, uniform=uniform)
    nc = _cache[key]

    xf = np.ascontiguousarray(x.reshape(rows, d))
    btab = _btab()
    in_maps = [
        {"x": xf[c * rows_per_core:(c + 1) * rows_per_core], "btab": btab}
        for c in range(N_CORES)
    ]
    if not uniform:
        wrow = np.ascontiguousarray(weight.reshape(1, d))
        for m in in_maps:
            m["wt"] = wrow
    res = bass_utils.run_bass_kernel_spmd(nc, in_maps,
                                          core_ids=list(range(N_CORES)),
                                          trace=_trace)
    y = np.concatenate([res.results[c]["y"] for c in range(N_CORES)], axis=0)
    out = y.reshape(x.shape)
    if _trace:
        return out, res
    return out

